# revision 8
# baseline (speedup 1.0000x reference)
"""Trainium2 Bass kernel for DifferentiableGMM log-likelihood.

Computes  out[n] = ln sum_k exp( -0.5*||(x[n]-mu[k])/s[k]||^2 - log|s[k]| + log w[k] )
for N=2,000,000 points, K=16 diagonal-covariance components, D=3.

Design (pure data parallel over 8 cores, 262144 points per core):
  The per-component log-prob is linear in the 6 per-point features
  F = [x1^2, x2^2, x3^2, x1, x2, x3]:
      lp[n,k] = sum_f W[f,k]*F[n,f] + c_k
  The HOST precomputes F in fp16, already transposed into the
  feature-major SBUF layout the PE needs (features on partitions,
  points on columns, 16 point-slots of 6 feature rows; slots 0-7 at
  partitions 0..47, slots 8-15 at partitions 64..111).  The device then
  only runs:
      PE  : lp matmul (fp16, 8 slots x 16 comps out partitions, 512-col
            blocks) and the 16-round "shifted ones" sum-matmul that
            reduces the 16 exp values per point across partitions.
      ACT : wide Exp (bias = c_k per out-partition, bf16 out to give the
            sum-matmul full dynamic range) and the final Ln.
      DVE : only stages the tiny constants.
  ACT is the bottleneck (~32k exp elems/lane/core); PE streams ~64k
  cycles; DMA moves 4MB in + 1MB out per core.  PSUM: 2x 3-bank m2
  buffers + 2x 1-bank sums accumulators = 8 banks.
  The device writes results in an interleaved order; the host gathers
  them back (free).
"""

import os
import numpy as np

K = 16
D = 3
EPS = 1e-6
N_CORES = 8

SLOTS = 16                 # point-slots per core (A: 0-7, B: 8-15)
CPS = 16384                # columns (points) per slot
NPC = SLOTS * CPS          # 262144 points per core
N_PAD = N_CORES * NPC      # 2097152
NMM = 2 * (CPS // 512)     # 64 lp matmuls (A/B interleaved) per core
# chunk = number of 512-col matmuls per PSUM buffer / Exp instruction
CHUNK_PLAN = [3] * 20 + [2] * 2    # sums to 64

_compiled_cache = {}


def _build_nc(use_f32r=True):
    # Force the ACT-table chooser to the one set holding Exp and Ln
    # together so no 1.3us table reloads happen mid-kernel.  Restored
    # after compile.
    import concourse.bacc as bacc
    import concourse.hw_specs as hw_specs
    import concourse.mybir as mybir
    import concourse.tile as tile
    from concourse._compat import get_trn_type

    f32 = mybir.dt.float32
    f16 = mybir.dt.float16
    bf16 = mybir.dt.bfloat16
    AF = mybir.ActivationFunctionType

    reps = int(os.environ.get("GMM_REPS", "1"))

    _orig_gat = bacc.get_activation_tables

    def _only_combined(arch):
        return {name: (fns if name == "natural_log_exp_and_others" else set())
                for name, fns in _orig_gat(arch).items()}

    bacc.get_activation_tables = _only_combined
    try:
        nc = bacc.Bacc(
            get_trn_type() or "TRN2",
            target_bir_lowering=False,
            debug=False,
            num_devices=N_CORES,
        )

        x_dram = nc.dram_tensor("x", [128, CPS], f16, kind="ExternalInput")
        w_dram = nc.dram_tensor("w", [128, 128], f16, kind="ExternalInput")
        ones_dram = nc.dram_tensor("ones", [128, 256], bf16, kind="ExternalInput")
        cvec_dram = nc.dram_tensor("cvec", [128, 1], f32, kind="ExternalInput")
        out_dram = nc.dram_tensor("out", [NPC], f32, kind="ExternalOutput")

        XCH = 2048                      # x-dma chunk columns
        NXD = CPS // XCH                # 8 dma chunks per rep

        with tile.TileContext(nc) as tc:
            with (
                tc.tile_pool(name="singles", bufs=1) as singles,
                tc.tile_pool(name="xin", bufs=2) as xin_pool,
                tc.tile_pool(name="etile", bufs=3) as e_pool,
                tc.tile_pool(name="osb", bufs=3) as out_pool,
                tc.tile_pool(name="mpsum", bufs=2, space="PSUM") as m_pool,
                tc.tile_pool(name="spsum", bufs=2, space="PSUM") as s_pool,
            ):
                # constants: DMA -> staging tile -> DVE copy, so consumers'
                # waits merge into compute-engine sem domains.
                W_st = singles.tile([128, 128], f16)
                ones_st = singles.tile([128, 256], bf16)
                cvec_st = singles.tile([128, 1], f32)
                nc.sync.dma_start(W_st[:], w_dram[:, :])
                nc.sync.dma_start(ones_st[:], ones_dram[:, :])
                nc.sync.dma_start(cvec_st[:], cvec_dram[:, :])
                Wt = singles.tile([128, 128], f16)
                ones_t = singles.tile([128, 256], bf16)
                cvec = singles.tile([128, 1], f32)
                nc.vector.tensor_copy(Wt[:], W_st[:])
                nc.vector.tensor_copy(ones_t[:], ones_st[:])
                nc.vector.tensor_copy(cvec[:], cvec_st[:])

                out_view = out_dram.ap().rearrange("(T p c) -> T p c", p=128, c=512)

                def main_body():
                    xch = [xin_pool.tile([128, XCH], f16, name=f"xch{d}")
                           for d in range(NXD)]
                    for d in range(NXD):
                        nc.sync.dma_start(xch[d][:], x_dram[:, XCH * d:XCH * (d + 1)])

                    sums_t = {}
                    m = 0
                    for width_mm in CHUNK_PLAN:
                        wcols = 512 * width_mm
                        m2_full = m_pool.tile([128, 512 * max(CHUNK_PLAN)], f32,
                                              name="m2")
                        m2 = m2_full[:, 0:wcols]
                        ms = range(m, m + width_mm)
                        for i, mm in enumerate(ms):
                            cb = mm // 2
                            col0 = 512 * cb
                            ch = xch[col0 // XCH]
                            cc = col0 % XCH
                            if mm % 2 == 0:
                                rows = slice(0, 48)
                            else:
                                rows = slice(64, 112)
                            nc.tensor.matmul(
                                m2[:, 512 * i:512 * i + 512],
                                Wt[rows, :],
                                ch[rows, cc:cc + 512],
                                start=True, stop=True,
                            )
                        e_full = e_pool.tile([128, 512 * max(CHUNK_PLAN)], bf16,
                                             name="e")
                        e = e_full[:, 0:wcols]
                        nc.scalar.activation(e, m2, AF.Exp,
                                             bias=cvec[:], scale=1.0)
                        for i, mm in enumerate(ms):
                            T, r = mm // 16, mm % 16
                            if r == 0:
                                sums_t[T] = s_pool.tile([128, 512], f32,
                                                        name="sums")
                            nc.tensor.matmul(
                                sums_t[T][:],
                                ones_t[:, 120 - 8 * r:248 - 8 * r],
                                e[:, 512 * i:512 * i + 512],
                                start=(r == 0), stop=(r == 15),
                            )
                            if r == 15:
                                out_sb = out_pool.tile([128, 512], f32)
                                nc.scalar.activation(out_sb[:], sums_t[T][:], AF.Ln)
                                nc.sync.dma_start(out_view[T], out_sb[:])
                        m += width_mm

                if reps == 1:
                    main_body()
                else:
                    with tc.For_i(0, reps, 1):
                        main_body()

        nc.compile()
    finally:
        bacc.get_activation_tables = _orig_gat
    return nc


def _host_constants(means, covariances, weights):
    """W [128,128] f16, cvec [128,1] f32, ones [128,256] bf16."""
    import concourse.mybir as mybir

    covp = covariances.astype(np.float64) + EPS
    mu = means.astype(np.float64)
    A = -0.5 / covp                              # [K,D] coeff of x^2
    B = mu / covp                                # [K,D] coeff of x
    c_k = (-0.5 * (mu * mu / covp).sum(1) - 0.5 * np.log(covp).sum(1)
           - 0.5 * D * np.log(2 * np.pi) + np.log(weights.astype(np.float64)))

    coefT = np.zeros((6, K), np.float64)
    coefT[0:3] = A.T
    coefT[3:6] = B.T
    wblk = np.zeros((48, 128), np.float64)
    for t in range(8):
        wblk[6 * t:6 * t + 6, 16 * t:16 * t + 16] = coefT
    W = np.zeros((128, 128), np.float64)
    W[0:48] = wblk
    W[64:112] = wblk

    cvec = np.tile(c_k, 8).reshape(128, 1).astype(np.float32)

    bf16 = mybir.dt.np(mybir.dt.bfloat16)
    ones = np.zeros((128, 256), np.float32)
    for t in range(8):
        ones[16 * t:16 * t + 16, 120 + t] = 1.0
    return W.astype(np.float16), cvec, ones.astype(bf16)


def _host_features(x_pad):
    """x_pad [N_PAD, 3] f32 -> per-core XT [8, 128, CPS] f16."""
    xs = x_pad.reshape(N_CORES, SLOTS, CPS, D)
    xf = xs.astype(np.float16)
    x2 = (xs.astype(np.float64) ** 2).astype(np.float16)
    feats = np.concatenate([x2, xf], axis=3)          # [8, 16, CPS, 6]
    featsT = feats.transpose(0, 1, 3, 2)              # [8, 16, 6, CPS]
    XT = np.zeros((N_CORES, 128, CPS), np.float16)
    XT[:, 0:48] = featsT[:, 0:8].reshape(N_CORES, 48, CPS)
    XT[:, 64:112] = featsT[:, 8:16].reshape(N_CORES, 48, CPS)
    return np.ascontiguousarray(XT)


def _output_permutation():
    """n[l]: point index for each linear output position l (per core)."""
    T, p, c = np.meshgrid(np.arange(NPC // 65536), np.arange(128),
                          np.arange(512), indexing="ij")
    r, t = p // 8, p % 8
    mm = 16 * T + r
    s = 8 * (mm % 2) + t
    j = 512 * (mm // 2) + c
    n = s * CPS + j
    return n.reshape(-1)


def build_in_maps(x, means, covariances, weights):
    x = np.ascontiguousarray(np.asarray(x, dtype=np.float32))
    means = np.asarray(means, dtype=np.float32)
    covariances = np.asarray(covariances, dtype=np.float32)
    weights = np.asarray(weights, dtype=np.float32).reshape(K)

    n = x.shape[0]
    x_pad = np.zeros((N_PAD, D), dtype=np.float32)
    x_pad[:n] = x
    XT = _host_features(x_pad)
    W, cvec, ones = _host_constants(means, covariances, weights)
    return [
        {"x": XT[c], "w": W, "ones": ones, "cvec": cvec}
        for c in range(N_CORES)
    ]


def kernel(x, means, covariances, weights):
    from concourse.bass_utils import run_bass_kernel_spmd

    n = np.asarray(x).shape[0]
    in_maps = build_in_maps(x, means, covariances, weights)

    key = "nc"
    if key not in _compiled_cache:
        _compiled_cache[key] = _build_nc()
    nc = _compiled_cache[key]

    res = run_bass_kernel_spmd(
        nc, in_maps, core_ids=list(range(N_CORES)),
        trace=bool(int(os.environ.get("GMM_TRACE", "0"))),
    )
    kernel.last_results = res

    perm = _output_permutation()
    out_pad = np.empty(N_PAD, dtype=np.float32)
    for c in range(N_CORES):
        raw = res.results[c]["out"].reshape(-1)
        out_pad[c * NPC + perm] = raw
    return out_pad[:n]


# revision 33
# speedup vs baseline: 1.3510x; 1.3510x over previous
"""Trainium2 Bass kernel for DifferentiableGMM log-likelihood.

Computes  out[n] = ln sum_k exp( -0.5*||(x[n]-mu[k])/s[k]||^2 - log|s[k]| + log w[k] )
for N=2,000,000 points, K=16 diagonal-covariance components, D=3.

Design (pure data parallel over 8 cores, 262144 points per core):
  The per-component log-prob is linear in the 6 per-point features
  F = [x1^2, x2^2, x3^2, x1, x2, x3]:
      lp[n,k] = sum_f W[f,k]*F[n,f] + c_k
  The HOST precomputes F in fp16, already transposed into the
  feature-major SBUF layout the PE needs (features on partitions,
  points on columns, 16 point-slots of 6 feature rows; slots 0-7 at
  partitions 0..47, slots 8-15 at partitions 64..111).  The device then
  only runs:
      PE  : lp matmul (fp16, 8 slots x 16 comps out partitions, 512-col
            blocks) and the 16-round "shifted ones" sum-matmul that
            reduces the 16 exp values per point across partitions.
      ACT : wide Exp (bias = c_k per out-partition, bf16 out to give the
            sum-matmul full dynamic range) and the final Ln.
      DVE : only stages the tiny constants.
  ACT is the bottleneck (~32k exp elems/lane/core); PE streams ~64k
  cycles; DMA moves 4MB in + 1MB out per core.  PSUM: 2x 3-bank m2
  buffers + 2x 1-bank sums accumulators = 8 banks.
  The device writes results in an interleaved order; the host gathers
  them back (free).
"""

import os
import numpy as np

K = 16
D = 3
EPS = 1e-6
N_CORES = 8

SLOTS = 16                 # point-slots per core (A: 0-7, B: 8-15)
CPS = 16384                # columns (points) per slot
NPC = SLOTS * CPS          # 262144 points per core
N_PAD = N_CORES * NPC      # 2097152
NMM = 2 * (CPS // 512)     # 64 lp matmuls (A/B interleaved) per core
# chunk = number of 512-col matmuls per PSUM buffer / Exp instruction
# (first chunk short so the pipeline starts on the first 512-col x-dma).
# Each chunk's lp matmuls all use the same slot group (A=even chunks,
# B=odd) so the stationary W does not reload between them.
CHUNK_PLAN = [2] + [3] * 20 + [2]    # sums to 64


def _block_map():
    """emission-order e-block m -> (group, colblock)."""
    return [(m % 2, m // 2) for m in range(NMM)]

_compiled_cache = {}


def _build_nc(use_f32r=True):
    # Force the ACT-table chooser to the one set holding Exp and Ln
    # together so no 1.3us table reloads happen mid-kernel.  Restored
    # after compile.
    import concourse.bacc as bacc
    import concourse.hw_specs as hw_specs
    import concourse.mybir as mybir
    import concourse.tile as tile
    from concourse._compat import get_trn_type

    f32 = mybir.dt.float32
    f16 = mybir.dt.float16
    bf16 = mybir.dt.bfloat16
    AF = mybir.ActivationFunctionType

    reps = int(os.environ.get("GMM_REPS", "1"))

    _orig_gat = bacc.get_activation_tables

    def _only_combined(arch):
        return {name: (fns if name == "natural_log_exp_and_others" else set())
                for name, fns in _orig_gat(arch).items()}

    bacc.get_activation_tables = _only_combined
    try:
        nc = bacc.Bacc(
            get_trn_type() or "TRN2",
            target_bir_lowering=False,
            debug=False,
            num_devices=N_CORES,
        )

        x_dram = nc.dram_tensor("x", [128, CPS], f16, kind="ExternalInput")
        w_dram = nc.dram_tensor("w", [128, 128], f16, kind="ExternalInput")
        ones_dram = nc.dram_tensor("ones", [128, 256], bf16, kind="ExternalInput")
        cvec_dram = nc.dram_tensor("cvec", [128, 1], f32, kind="ExternalInput")
        out_dram = nc.dram_tensor("out", [NPC], f32, kind="ExternalOutput")

        # x-dma chunk columns: all on the Sync queue, which executes
        # transfers IN ORDER — a small first chunk lands early so compute
        # starts ~8us in while the rest stream behind it.  W/cvec (needed
        # by the first matmul/exp) go on the same queue ahead of chunk 0;
        # only `ones` (needed ~2us later) rides the gpsimd SWDGE queue.
        XCHUNKS = [512, 1536] + [2048] * 7

        with tile.TileContext(nc) as tc:
            with (
                tc.tile_pool(name="singles", bufs=1) as singles,
                tc.tile_pool(name="xin", bufs=2) as xin_pool,
                tc.tile_pool(name="etile", bufs=3) as e_pool,
                tc.tile_pool(name="osb", bufs=3) as out_pool,
                tc.tile_pool(name="mpsum", bufs=2, space="PSUM") as m_pool,
                tc.tile_pool(name="spsum", bufs=2, space="PSUM") as s_pool,
            ):
                # ACT-table preload: a dummy Exp on a zeroed scratch tile
                # makes the compiler hoist the 1.3us table load to t=0,
                # overlapping the startup DMAs.
                scr_in = singles.tile([128, 1], f32)
                scr_out = singles.tile([128, 1], f32)
                nc.vector.memset(scr_in[:], 0.0)
                nc.scalar.activation(scr_out[:], scr_in[:], AF.Exp)

                # constants: DMA -> staging tile -> DVE copy, so consumers'
                # waits merge into compute-engine sem domains.
                W_st = singles.tile([128, 128], f16)
                ones_st = singles.tile([128, 256], bf16)
                cvec_st = singles.tile([128, 1], f32)
                # consts ride the gpsimd SWDGE queue: tiny transfers that
                # land by ~9us without delaying chunk 0 on the Sync queue.
                nc.gpsimd.dma_start(W_st[:], w_dram[:, :])
                nc.gpsimd.dma_start(cvec_st[:], cvec_dram[:, :])
                nc.gpsimd.dma_start(ones_st[:], ones_dram[:, :])
                Wt = singles.tile([128, 128], f16)
                ones_t = singles.tile([128, 256], bf16)
                cvec = singles.tile([128, 1], f32)
                nc.vector.tensor_copy(Wt[:], W_st[:])
                nc.vector.tensor_copy(ones_t[:], ones_st[:])
                nc.vector.tensor_copy(cvec[:], cvec_st[:])

                out_view = out_dram.ap().rearrange("(T p c) -> T p c", p=128, c=512)

                xoff = np.cumsum([0] + XCHUNKS)

                def main_body():
                    xch = []
                    for d, w in enumerate(XCHUNKS):
                        t = xin_pool.tile([128, w], f16, name=f"xch{d}")
                        nc.sync.dma_start(t[:], x_dram[:, xoff[d]:xoff[d + 1]])
                        xch.append(t)

                    def xcols(col0, ncols):
                        d = int(np.searchsorted(xoff, col0, side="right")) - 1
                        assert col0 + ncols <= xoff[d + 1]
                        return xch[d], col0 - xoff[d]

                    blocks = _block_map()
                    sums_t = {}
                    m = 0
                    for width_mm in CHUNK_PLAN:
                        wcols = 512 * width_mm
                        m2_full = m_pool.tile([128, 512 * max(CHUNK_PLAN)], f32,
                                              name="m2")
                        m2 = m2_full[:, 0:wcols]
                        ms = range(m, m + width_mm)
                        for i, mm in enumerate(ms):
                            g, cb = blocks[mm]
                            ch, cc = xcols(512 * cb, 512)
                            rows = slice(0, 48) if g == 0 else slice(64, 112)
                            nc.tensor.matmul(
                                m2[:, 512 * i:512 * i + 512],
                                Wt[rows, :],
                                ch[rows, cc:cc + 512],
                                start=True, stop=True,
                            )
                        e_full = e_pool.tile([128, 512 * max(CHUNK_PLAN)], bf16,
                                             name="e")
                        e = e_full[:, 0:wcols]
                        nc.scalar.activation(e, m2, AF.Exp,
                                             bias=cvec[:], scale=1.0)
                        for i, mm in enumerate(ms):
                            T, r = mm // 16, mm % 16
                            if r == 0:
                                sums_t[T] = s_pool.tile([128, 512], f32,
                                                        name="sums")
                            nc.tensor.matmul(
                                sums_t[T][:],
                                ones_t[:, 120 - 8 * r:248 - 8 * r],
                                e[:, 512 * i:512 * i + 512],
                                start=(r == 0), stop=(r == 15),
                            )
                            if r == 15:
                                out_sb = out_pool.tile([128, 512], f32)
                                nc.scalar.activation(out_sb[:], sums_t[T][:], AF.Ln)
                                nc.sync.dma_start(out_view[T], out_sb[:])
                        m += width_mm

                if reps == 1:
                    main_body()
                else:
                    with tc.For_i(0, reps, 1):
                        main_body()

        nc.compile()
    finally:
        bacc.get_activation_tables = _orig_gat
    return nc


def _host_constants(means, covariances, weights):
    """W [128,128] f16, cvec [128,1] f32, ones [128,256] bf16."""
    import concourse.mybir as mybir

    covp = covariances.astype(np.float64) + EPS
    mu = means.astype(np.float64)
    A = -0.5 / covp                              # [K,D] coeff of x^2
    B = mu / covp                                # [K,D] coeff of x
    c_k = (-0.5 * (mu * mu / covp).sum(1) - 0.5 * np.log(covp).sum(1)
           - 0.5 * D * np.log(2 * np.pi) + np.log(weights.astype(np.float64)))

    coefT = np.zeros((6, K), np.float64)
    coefT[0:3] = A.T
    coefT[3:6] = B.T
    wblk = np.zeros((48, 128), np.float64)
    for t in range(8):
        wblk[6 * t:6 * t + 6, 16 * t:16 * t + 16] = coefT
    W = np.zeros((128, 128), np.float64)
    W[0:48] = wblk
    W[64:112] = wblk

    cvec = np.tile(c_k, 8).reshape(128, 1).astype(np.float32)

    bf16 = mybir.dt.np(mybir.dt.bfloat16)
    ones = np.zeros((128, 256), np.float32)
    for t in range(8):
        ones[16 * t:16 * t + 16, 120 + t] = 1.0
    return W.astype(np.float16), cvec, ones.astype(bf16)


def _host_features(x_pad):
    """x_pad [N_PAD, 3] f32 -> per-core XT [8, 128, CPS] f16."""
    xs = x_pad.reshape(N_CORES, SLOTS, CPS, D)
    xf = xs.astype(np.float16)
    x2 = (xs.astype(np.float64) ** 2).astype(np.float16)
    feats = np.concatenate([x2, xf], axis=3)          # [8, 16, CPS, 6]
    featsT = feats.transpose(0, 1, 3, 2)              # [8, 16, 6, CPS]
    XT = np.zeros((N_CORES, 128, CPS), np.float16)
    XT[:, 0:48] = featsT[:, 0:8].reshape(N_CORES, 48, CPS)
    XT[:, 64:112] = featsT[:, 8:16].reshape(N_CORES, 48, CPS)
    return np.ascontiguousarray(XT)


def _output_permutation():
    """n[l]: point index for each linear output position l (per core)."""
    blocks = np.array(_block_map())                       # [64, 2]
    T, p, c = np.meshgrid(np.arange(NPC // 65536), np.arange(128),
                          np.arange(512), indexing="ij")
    r, t = p // 8, p % 8
    mm = 16 * T + r
    g, cb = blocks[mm, 0], blocks[mm, 1]
    s = 8 * g + t
    j = 512 * cb + c
    n = s * CPS + j
    return n.reshape(-1)


def build_in_maps(x, means, covariances, weights):
    x = np.ascontiguousarray(np.asarray(x, dtype=np.float32))
    means = np.asarray(means, dtype=np.float32)
    covariances = np.asarray(covariances, dtype=np.float32)
    weights = np.asarray(weights, dtype=np.float32).reshape(K)

    n = x.shape[0]
    x_pad = np.zeros((N_PAD, D), dtype=np.float32)
    x_pad[:n] = x
    XT = _host_features(x_pad)
    W, cvec, ones = _host_constants(means, covariances, weights)
    return [
        {"x": XT[c], "w": W, "ones": ones, "cvec": cvec}
        for c in range(N_CORES)
    ]


def kernel(x, means, covariances, weights):
    from concourse.bass_utils import run_bass_kernel_spmd

    n = np.asarray(x).shape[0]
    in_maps = build_in_maps(x, means, covariances, weights)

    key = "nc"
    if key not in _compiled_cache:
        _compiled_cache[key] = _build_nc()
    nc = _compiled_cache[key]

    res = run_bass_kernel_spmd(
        nc, in_maps, core_ids=list(range(N_CORES)),
        trace=bool(int(os.environ.get("GMM_TRACE", "0"))),
    )
    kernel.last_results = res

    perm = _output_permutation()
    out_pad = np.empty(N_PAD, dtype=np.float32)
    for c in range(N_CORES):
        raw = res.results[c]["out"].reshape(-1)
        out_pad[c * NPC + perm] = raw
    return out_pad[:n]


# revision 35
# speedup vs baseline: 9.0677x; 6.7118x over previous
"""Trainium2 Bass kernel for DifferentiableGMM log-likelihood.

Computes  out[n] = ln sum_k exp( -0.5*||(x[n]-mu[k])/s[k]||^2 - log|s[k]| + log w[k] )
for N=2,000,000 points, K=16 diagonal-covariance components, D=3.

Design (pure data parallel over 8 cores, 262144 points per core):
  The per-component log-prob is linear in the 6 per-point features
  F = [x1^2, x2^2, x3^2, x1, x2, x3]:
      lp[n,k] = sum_f W[f,k]*F[n,f] + c_k
  The HOST precomputes F in fp16, already transposed into the
  feature-major SBUF layout the PE needs (features on partitions,
  points on columns, 16 point-slots of 6 feature rows; slots 0-7 at
  partitions 0..47, slots 8-15 at partitions 64..111).  The device then
  only runs:
      PE  : lp matmul (fp16, 8 slots x 16 comps out partitions, 512-col
            blocks) and the 16-round "shifted ones" sum-matmul that
            reduces the 16 exp values per point across partitions.
      ACT : wide Exp (bias = c_k per out-partition, bf16 out to give the
            sum-matmul full dynamic range) and the final Ln.
      DVE : only stages the tiny constants.
  ACT is the bottleneck (~32k exp elems/lane/core); PE streams ~64k
  cycles; DMA moves 4MB in + 1MB out per core.  PSUM: 2x 3-bank m2
  buffers + 2x 1-bank sums accumulators = 8 banks.
  The device writes results in an interleaved order; the host gathers
  them back (free).
"""

import os
import numpy as np

K = 16
D = 3
EPS = 1e-6
N_CORES = 8

SLOTS = 16                 # point-slots per core (A: 0-7, B: 8-15)
CPS = 16384                # columns (points) per slot
NPC = SLOTS * CPS          # 262144 points per core
N_PAD = N_CORES * NPC      # 2097152
NMM = 2 * (CPS // 512)     # 64 lp matmuls (A/B interleaved) per core
# chunk = number of 512-col matmuls per PSUM buffer / Exp instruction
# (first chunk short so the pipeline starts on the first 512-col x-dma).
# Each chunk's lp matmuls all use the same slot group (A=even chunks,
# B=odd) so the stationary W does not reload between them.
CHUNK_PLAN = [2] + [3] * 20 + [2]    # sums to 64


def _block_map():
    """emission-order e-block m -> (group, colblock)."""
    return [(m % 2, m // 2) for m in range(NMM)]

_compiled_cache = {}


def _build_nc(use_f32r=True):
    # Force the ACT-table chooser to the one set holding Exp and Ln
    # together so no 1.3us table reloads happen mid-kernel.  Restored
    # after compile.
    import concourse.bacc as bacc
    import concourse.hw_specs as hw_specs
    import concourse.mybir as mybir
    import concourse.tile as tile
    from concourse._compat import get_trn_type

    f32 = mybir.dt.float32
    f16 = mybir.dt.float16
    bf16 = mybir.dt.bfloat16
    AF = mybir.ActivationFunctionType

    reps = int(os.environ.get("GMM_REPS", "1"))

    _orig_gat = bacc.get_activation_tables

    def _only_combined(arch):
        return {name: (fns if name == "natural_log_exp_and_others" else set())
                for name, fns in _orig_gat(arch).items()}

    bacc.get_activation_tables = _only_combined
    try:
        nc = bacc.Bacc(
            get_trn_type() or "TRN2",
            target_bir_lowering=False,
            debug=False,
            num_devices=N_CORES,
        )

        x_dram = nc.dram_tensor("x", [128, CPS], f16, kind="ExternalInput")
        w_dram = nc.dram_tensor("w", [128, 128], f16, kind="ExternalInput")
        ones_dram = nc.dram_tensor("ones", [128, 256], bf16, kind="ExternalInput")
        cvec_dram = nc.dram_tensor("cvec", [128, 1], f32, kind="ExternalInput")
        out_dram = nc.dram_tensor("out", [NPC], f32, kind="ExternalOutput")

        # x-dma chunk columns: all on the Sync queue, which executes
        # transfers IN ORDER — a small first chunk lands early so compute
        # starts ~8us in while the rest stream behind it.  W/cvec (needed
        # by the first matmul/exp) go on the same queue ahead of chunk 0;
        # only `ones` (needed ~2us later) rides the gpsimd SWDGE queue.
        XCHUNKS = [512, 1536] + [2048] * 7

        with tile.TileContext(nc) as tc:
            with (
                tc.tile_pool(name="singles", bufs=1) as singles,
                tc.tile_pool(name="xin", bufs=2) as xin_pool,
                tc.tile_pool(name="etile", bufs=3) as e_pool,
                tc.tile_pool(name="osb", bufs=3) as out_pool,
                tc.tile_pool(name="mpsum", bufs=2, space="PSUM") as m_pool,
                tc.tile_pool(name="spsum", bufs=2, space="PSUM") as s_pool,
            ):
                # ACT-table preload: a dummy Exp on a zeroed scratch tile
                # makes the compiler hoist the 1.3us table load to t=0,
                # overlapping the startup DMAs.
                scr_in = singles.tile([128, 1], f32)
                scr_out = singles.tile([128, 1], f32)
                nc.vector.memset(scr_in[:], 0.0)
                nc.scalar.activation(scr_out[:], scr_in[:], AF.Exp)

                # constants: DMA -> staging tile -> DVE copy, so consumers'
                # waits merge into compute-engine sem domains.
                W_st = singles.tile([128, 128], f16)
                ones_st = singles.tile([128, 256], bf16)
                cvec_st = singles.tile([128, 1], f32)
                # consts ride the gpsimd SWDGE queue: tiny transfers that
                # land by ~9us without delaying chunk 0 on the Sync queue.
                nc.gpsimd.dma_start(W_st[:], w_dram[:, :])
                nc.gpsimd.dma_start(cvec_st[:], cvec_dram[:, :])
                nc.gpsimd.dma_start(ones_st[:], ones_dram[:, :])
                Wt = singles.tile([128, 128], f16)
                ones_t = singles.tile([128, 256], bf16)
                cvec = singles.tile([128, 1], f32)
                nc.vector.tensor_copy(Wt[:], W_st[:])
                nc.vector.tensor_copy(ones_t[:], ones_st[:])
                nc.vector.tensor_copy(cvec[:], cvec_st[:])

                out_view = out_dram.ap().rearrange("(T p c) -> T p c", p=128, c=512)

                xoff = np.cumsum([0] + XCHUNKS)

                def main_body():
                    xch = []
                    for d, w in enumerate(XCHUNKS):
                        t = xin_pool.tile([128, w], f16, name=f"xch{d}")
                        nc.sync.dma_start(t[:], x_dram[:, xoff[d]:xoff[d + 1]])
                        xch.append(t)

                    def xcols(col0, ncols):
                        d = int(np.searchsorted(xoff, col0, side="right")) - 1
                        assert col0 + ncols <= xoff[d + 1]
                        return xch[d], col0 - xoff[d]

                    blocks = _block_map()
                    sums_t = {}
                    m = 0
                    for width_mm in CHUNK_PLAN:
                        wcols = 512 * width_mm
                        m2_full = m_pool.tile([128, 512 * max(CHUNK_PLAN)], f32,
                                              name="m2")
                        m2 = m2_full[:, 0:wcols]
                        ms = range(m, m + width_mm)
                        for i, mm in enumerate(ms):
                            g, cb = blocks[mm]
                            ch, cc = xcols(512 * cb, 512)
                            rows = slice(0, 48) if g == 0 else slice(64, 112)
                            nc.tensor.matmul(
                                m2[:, 512 * i:512 * i + 512],
                                Wt[rows, :],
                                ch[rows, cc:cc + 512],
                                start=True, stop=True,
                            )
                        e_full = e_pool.tile([128, 512 * max(CHUNK_PLAN)], bf16,
                                             name="e")
                        e = e_full[:, 0:wcols]
                        nc.scalar.activation(e, m2, AF.Exp,
                                             bias=cvec[:], scale=1.0)
                        for i, mm in enumerate(ms):
                            T, r = mm // 16, mm % 16
                            if r == 0:
                                sums_t[T] = s_pool.tile([128, 512], f32,
                                                        name="sums")
                            nc.tensor.matmul(
                                sums_t[T][:],
                                ones_t[:, 120 - 8 * r:248 - 8 * r],
                                e[:, 512 * i:512 * i + 512],
                                start=(r == 0), stop=(r == 15),
                            )
                            if r == 15:
                                out_sb = out_pool.tile([128, 512], f32)
                                nc.scalar.activation(out_sb[:], sums_t[T][:], AF.Ln)
                                # gpsimd queue: keeps the in-order Sync queue
                                # free for the next iteration's x chunks
                                nc.gpsimd.dma_start(out_view[T], out_sb[:])
                        m += width_mm

                if reps == 1:
                    main_body()
                else:
                    # unroll 8 bodies per hardware-loop pass: the For_i
                    # boundary is a full cross-queue barrier (~9.5us), but
                    # bodies within a pass pipeline back-to-back.
                    UNROLL = 8
                    assert reps % UNROLL == 0, reps
                    with tc.For_i(0, reps // UNROLL, 1):
                        for _ in range(UNROLL):
                            main_body()

        nc.compile()
    finally:
        bacc.get_activation_tables = _orig_gat
    return nc


def _host_constants(means, covariances, weights):
    """W [128,128] f16, cvec [128,1] f32, ones [128,256] bf16."""
    import concourse.mybir as mybir

    covp = covariances.astype(np.float64) + EPS
    mu = means.astype(np.float64)
    A = -0.5 / covp                              # [K,D] coeff of x^2
    B = mu / covp                                # [K,D] coeff of x
    c_k = (-0.5 * (mu * mu / covp).sum(1) - 0.5 * np.log(covp).sum(1)
           - 0.5 * D * np.log(2 * np.pi) + np.log(weights.astype(np.float64)))

    coefT = np.zeros((6, K), np.float64)
    coefT[0:3] = A.T
    coefT[3:6] = B.T
    wblk = np.zeros((48, 128), np.float64)
    for t in range(8):
        wblk[6 * t:6 * t + 6, 16 * t:16 * t + 16] = coefT
    W = np.zeros((128, 128), np.float64)
    W[0:48] = wblk
    W[64:112] = wblk

    cvec = np.tile(c_k, 8).reshape(128, 1).astype(np.float32)

    bf16 = mybir.dt.np(mybir.dt.bfloat16)
    ones = np.zeros((128, 256), np.float32)
    for t in range(8):
        ones[16 * t:16 * t + 16, 120 + t] = 1.0
    return W.astype(np.float16), cvec, ones.astype(bf16)


def _host_features(x_pad):
    """x_pad [N_PAD, 3] f32 -> per-core XT [8, 128, CPS] f16."""
    xs = x_pad.reshape(N_CORES, SLOTS, CPS, D)
    xf = xs.astype(np.float16)
    x2 = (xs.astype(np.float64) ** 2).astype(np.float16)
    feats = np.concatenate([x2, xf], axis=3)          # [8, 16, CPS, 6]
    featsT = feats.transpose(0, 1, 3, 2)              # [8, 16, 6, CPS]
    XT = np.zeros((N_CORES, 128, CPS), np.float16)
    XT[:, 0:48] = featsT[:, 0:8].reshape(N_CORES, 48, CPS)
    XT[:, 64:112] = featsT[:, 8:16].reshape(N_CORES, 48, CPS)
    return np.ascontiguousarray(XT)


def _output_permutation():
    """n[l]: point index for each linear output position l (per core)."""
    blocks = np.array(_block_map())                       # [64, 2]
    T, p, c = np.meshgrid(np.arange(NPC // 65536), np.arange(128),
                          np.arange(512), indexing="ij")
    r, t = p // 8, p % 8
    mm = 16 * T + r
    g, cb = blocks[mm, 0], blocks[mm, 1]
    s = 8 * g + t
    j = 512 * cb + c
    n = s * CPS + j
    return n.reshape(-1)


def build_in_maps(x, means, covariances, weights):
    x = np.ascontiguousarray(np.asarray(x, dtype=np.float32))
    means = np.asarray(means, dtype=np.float32)
    covariances = np.asarray(covariances, dtype=np.float32)
    weights = np.asarray(weights, dtype=np.float32).reshape(K)

    n = x.shape[0]
    x_pad = np.zeros((N_PAD, D), dtype=np.float32)
    x_pad[:n] = x
    XT = _host_features(x_pad)
    W, cvec, ones = _host_constants(means, covariances, weights)
    return [
        {"x": XT[c], "w": W, "ones": ones, "cvec": cvec}
        for c in range(N_CORES)
    ]


def kernel(x, means, covariances, weights):
    from concourse.bass_utils import run_bass_kernel_spmd

    n = np.asarray(x).shape[0]
    in_maps = build_in_maps(x, means, covariances, weights)

    key = "nc"
    if key not in _compiled_cache:
        _compiled_cache[key] = _build_nc()
    nc = _compiled_cache[key]

    res = run_bass_kernel_spmd(
        nc, in_maps, core_ids=list(range(N_CORES)),
        trace=bool(int(os.environ.get("GMM_TRACE", "0"))),
    )
    kernel.last_results = res

    perm = _output_permutation()
    out_pad = np.empty(N_PAD, dtype=np.float32)
    for c in range(N_CORES):
        raw = res.results[c]["out"].reshape(-1)
        out_pad[c * NPC + perm] = raw
    return out_pad[:n]
